# revision 3
# baseline (speedup 1.0000x reference)
"""Trainium2 Bass kernel v2: 10-layer LSTM (D=25) + FC(7) + softmax.

Data-parallel over batch (64 per core x 8). Anti-diagonal wavefront over
(layer, time); all tensors fp16 except PSUM/c-sums kept fp32 where free.

Per window w, per layer-group (0-4 / 5-9):
  - per layer: 2 fp16 matmuls (input-part K=26 incl. ones-row bias,
    recurrent-part K=25) into one PSUM tile [121, 320] with gate order
    i@0:25 f@32:57 o@64:89 g@96:121 (layers side by side in free dim).
  - sigmoid over psum[0:89] -> sg (fp16), tanh over psum[96:121] -> gc
    rows 0:25 (g-tilde, parity block of c storage).
  - DVE fp16 (2x mode): v = sig_f * c_prev ; u = sig_i * g_tilde ;
    c = u + v ; h = sig_o * tanh(c)  [tanh on Act engine]
  - h written to flat ring hbuf[w%4] at cols 64*l (ones row 25 preset),
    which is directly the next window's matmul rhs: layer l reads
    hbuf[w-1] cols 64(l-1) (h_{l-1}(t)) and 64l (h_l(t-1)).
Tail: FC + exp + class-sum matmuls (4-band packing), softmax divide on host.
"""
import sys, os

for _p in ("/opt/trn_rl_repo", "/root/.axon_site/_ro/trn_rl_repo"):
    if os.path.isdir(_p) and _p not in sys.path:
        sys.path.insert(0, _p)

import numpy as np
import concourse.bass as bass
import concourse.mybir as mybir
from concourse.tile import TileContext
from concourse.bass_utils import run_bass_kernel_spmd

F32 = mybir.dt.float32
F16 = mybir.dt.float16
AF = mybir.ActivationFunctionType

T, B, D, L, NCLS = 2048, 512, 25, 10, 7
NC = 8
BS = B // NC          # 64 batch per core
NB = 4                # hbuf ring depth
GROUPS = ((0, 5), (5, 10)) if os.environ.get("ABL_NG2") else ((0, 4), (4, 7), (7, 10))
F = 5 * BS            # 320 free per group
M = 121               # psum rows: i@0:25 f@32:57 o@64:89 g@96:121

# torch gate order in W_ih rows: i, f, g, o -> psum col bands
_GBAND = {0: 0, 1: 32, 3: 64, 2: 96}   # W row-block idx -> lhsT col offset


# ---------------------------------------------------------------- weights
def _pack_weights(W_ih, W_hh, b, fc_w, fc_b):
    """fp16 weight blob [26, ncols] + col table; selector [103, 4]."""
    mats = []
    cols = {}

    def add(key, m26):
        cols[key] = sum(x.shape[1] for x in mats)
        mats.append(m26)

    for l in range(L):
        win = np.zeros((26, M), np.float32)
        wrec = np.zeros((26, M), np.float32)
        for gi in range(4):
            c0 = _GBAND[gi]
            win[0:25, c0:c0 + 25] = W_ih[l][25 * gi:25 * gi + 25].T
            win[25, c0:c0 + 25] = b[l][25 * gi:25 * gi + 25]
            wrec[0:25, c0:c0 + 25] = W_hh[l][25 * gi:25 * gi + 25].T
        add(("win", l), win)
        add(("wrec", l), wrec)

    for g in range(4):
        m = np.zeros((26, 103), np.float32)
        m[0:25, 32 * g:32 * g + 7] = fc_w.T
        m[25, 32 * g:32 * g + 7] = fc_b
        add(("fc", g), m)

    ncols = sum(m.shape[1] for m in mats)
    blob = np.zeros((26, ncols), np.float16)
    c = 0
    for m in mats:
        blob[:, c:c + m.shape[1]] = m.astype(np.float16)
        c += m.shape[1]

    sel = np.zeros((103, 4), np.float16)
    for g in range(4):
        sel[32 * g:32 * g + 7, g] = 1.0
    return blob, cols, sel


_WFCTR = [0]


def _split_excess_waits(nc):
    """Walrus allows 1 sync-wait per instruction (2 for EventSemaphore).
    Hoist extras onto preceding same-engine NOPs."""
    for fn in nc.m.functions:
        for blk in fn.blocks:
            insts = list(blk.instructions)
            out = []
            changed = False
            for inst in insts:
                si = inst.sync_info
                cap = 2 if type(inst).__name__ == "InstEventSemaphore" else 1
                if si is not None and si.on_wait is not None and len(si.on_wait) > cap:
                    waits = list(si.on_wait)
                    extra, keep = waits[:-cap], waits[-cap:]
                    for wt in extra:
                        _WFCTR[0] += 1
                        out.append(mybir.InstNoOp(
                            name=f"I-waitfix-{_WFCTR[0]}", opcode="NoOp",
                            engine=inst.engine, ins=[], outs=[],
                            sync_info=mybir.SyncInfo(on_wait=[wt], on_update=[]),
                        ))
                    inst.sync_info = mybir.SyncInfo(
                        on_wait=keep, on_update=list(si.on_update))
                    changed = True
                out.append(inst)
            if changed:
                blk.instructions = out


# ---------------------------------------------------------------- program
_PROG_CACHE = {}


def _build_program(Tn, wcols, nwcols, patch_waits=True):
    nc = bass.Bass()
    NW = Tn + L - 1
    NCHUNK = (Tn * BS) // 512
    NGRP = NCHUNK // 4

    xT = nc.declare_dram_parameter("xT", [26, Tn * BS], F16, isOutput=False)
    wpack = nc.declare_dram_parameter("wpack", [26, nwcols], F16, isOutput=False)
    selp = nc.declare_dram_parameter("selp", [103, 4], F16, isOutput=False)
    c0T = nc.declare_dram_parameter("c0T", [25, L * BS], F16, isOutput=False)
    h0T = nc.declare_dram_parameter("h0T", [L, 25, BS], F16, isOutput=False)
    onesd = nc.declare_dram_parameter("onesd", [1, NB * 640], F16, isOutput=False)
    exp_out = nc.declare_dram_parameter("exp_out", [NGRP, 103, 512], F16, isOutput=True)
    sum_out = nc.declare_dram_parameter("sum_out", [NGRP, 4, 512], F32, isOutput=True)
    h9d = nc.dram_tensor("h9d", [25, Tn * BS], F16)

    def active(w):
        return range(max(0, w - Tn + 1), min(L - 1, w) + 1)

    def hcol(w, l):
        return (w % NB) * 640 + 64 * l

    def xcol(w):
        return (w % 128) * BS

    with TileContext(nc) as tc:
        import os as _os
        _wb = int(_os.environ.get("ABL_BUFS", 3))
        _pb = int(_os.environ.get("ABL_PSUM", 2))
        _noh9 = _os.environ.get("ABL_NO_H9")
        _notl = _os.environ.get("ABL_NO_TAIL_OPS")
        with (
            tc.tile_pool(name="pers", bufs=1) as pers,
            tc.tile_pool(name="work", bufs=_wb) as wp,
            tc.tile_pool(name="gps", bufs=(2 if _os.environ.get("ABL_GFIRST") else _pb), space="PSUM") as gps,
        ):
            hbuf = pers.tile([32, NB * 640], F16)
            xstage = pers.tile([26, 2 * 64 * BS], F16)
            wsb = pers.tile([26, nwcols], F16)
            selsb = pers.tile([103, 4], F16)
            gc = [pers.tile([57, 2 * F], F16, name=f"gc{g}") for g in range(len(GROUPS))]

            # ---- init
            nc.sync.dma_start(out=wsb[:, :], in_=wpack[:, :])
            nc.sync.dma_start(out=selsb[:, :], in_=selp[:, :])
            nc.sync.dma_start(out=hbuf[25:26, :], in_=onesd[:, :])
            for l in range(L):
                nc.sync.dma_start(
                    out=hbuf[0:25, ((l - 1) % NB) * 640 + 64 * l:
                             ((l - 1) % NB) * 640 + 64 * l + 64],
                    in_=h0T[l, :, :])
            for gi, (g0, g1) in enumerate(GROUPS):
                for l in range(g0, g1):
                    nc.sync.dma_start(
                        out=gc[gi][32:57, ((l + 1) % 2) * F + 64 * (l - g0):
                                   ((l + 1) % 2) * F + 64 * (l - g0) + 64],
                        in_=c0T[:, l * BS:(l + 1) * BS])
            for blk in range(min(2, (Tn + 63) // 64)):
                ce = min((blk + 1) * 64 * BS, Tn * BS)
                nc.sync.dma_start(
                    out=xstage[:, blk * 64 * BS:ce],
                    in_=xT[:, blk * 64 * BS:ce])

            # ---- wavefront
            for w in range(NW):
                act = list(active(w))
                pcur, pprev = (w % 2) * F, ((w + 1) % 2) * F
                gdat = []
                for gi, (g0, g1) in enumerate(GROUPS):
                    lo, hi = max(g0, act[0]), min(g1 - 1, act[-1])
                    if lo > hi:
                        continue
                    co = (lo - g0) * BS
                    gw = (hi - lo + 1) * BS
                    ps = gps.tile([M, F], F32, tag=f"g{gi}")
                    _gfirst = _os.environ.get("ABL_GFIRST")
                    psg = None
                    if _gfirst:
                        psg = gps.tile([25, F], F32, tag=f"psg{gi}")
                        for l in range(lo, hi + 1):
                            oc = (l - g0) * BS
                            wi = wcols[("win", l)]
                            wr = wcols[("wrec", l)]
                            rhs_in = (xstage[0:26, xcol(w):xcol(w) + BS] if l == 0
                                      else hbuf[0:26, hcol(w - 1, l - 1):hcol(w - 1, l - 1) + BS])
                            nc.tensor.matmul(
                                psg[0:25, oc:oc + BS], wsb[0:26, wi + 96:wi + 121],
                                rhs_in, start=True, stop=False)
                            nc.tensor.matmul(
                                psg[0:25, oc:oc + BS], wsb[0:25, wr + 96:wr + 121],
                                hbuf[0:25, hcol(w - 1, l):hcol(w - 1, l) + BS],
                                start=False, stop=True)
                    for l in range(lo, hi + 1):
                        oc = (l - g0) * BS
                        wi = wcols[("win", l)]
                        wr = wcols[("wrec", l)]
                        rhs_in = (xstage[0:26, xcol(w):xcol(w) + BS] if l == 0
                                  else hbuf[0:26, hcol(w - 1, l - 1):hcol(w - 1, l - 1) + BS])
                        _mw = 89 if _gfirst else M
                        nc.tensor.matmul(
                            ps[0:_mw, oc:oc + BS], wsb[0:26, wi:wi + _mw],
                            rhs_in, start=True, stop=False)
                        nc.tensor.matmul(
                            ps[0:_mw, oc:oc + BS], wsb[0:25, wr:wr + _mw],
                            hbuf[0:25, hcol(w - 1, l):hcol(w - 1, l) + BS],
                            start=False, stop=True)
                    sg = wp.tile([89, F], F16, tag=f"sg{gi}")
                    uv = wp.tile([57, 2 * F], F16, tag=f"uv{gi}")
                    th = wp.tile([89, F], F16, tag=f"th{gi}")
                    gdat.append((gi, lo, co, gw, ps, sg, uv, th, psg))

                # stage-interleaved emission across groups (keeps each
                # engine queue free of cross-stage head-of-line stalls)
                if _os.environ.get("ABL_GFIRST"):
                    for gi, lo, co, gw, ps, sg, uv, th, psg in gdat:
                        nc.scalar.activation(gc[gi][0:25, pprev + co:pprev + co + gw],
                                             psg[0:25, co:co + gw], AF.Tanh)
                    for gi, lo, co, gw, ps, sg, uv, th, psg in gdat:
                        nc.scalar.activation(sg[0:89, co:co + gw], ps[0:89, co:co + gw], AF.Sigmoid)
                else:
                    for gi, lo, co, gw, ps, sg, uv, th, psg in gdat:
                        nc.scalar.activation(sg[0:89, co:co + gw], ps[0:89, co:co + gw], AF.Sigmoid)
                    for gi, lo, co, gw, ps, sg, uv, th, psg in gdat:
                        nc.scalar.activation(gc[gi][0:25, pprev + co:pprev + co + gw],
                                             ps[96:121, co:co + gw], AF.Tanh)
                for gi, lo, co, gw, ps, sg, uv, th, psg in gdat:
                    # v = sig_f * c_prev
                    nc.vector.tensor_mul(uv[32:57, F + co:F + co + gw],
                                         sg[32:57, co:co + gw],
                                         gc[gi][32:57, pprev + co:pprev + co + gw])
                for gi, lo, co, gw, ps, sg, uv, th, psg in gdat:
                    # u = sig_i * g_tilde
                    nc.vector.tensor_mul(uv[32:57, co:co + gw],
                                         sg[0:25, co:co + gw],
                                         gc[gi][0:25, pprev + co:pprev + co + gw])
                for gi, lo, co, gw, ps, sg, uv, th, psg in gdat:
                    nc.vector.tensor_add(gc[gi][32:57, pcur + co:pcur + co + gw],
                                         uv[32:57, co:co + gw],
                                         uv[32:57, F + co:F + co + gw])
                if not _notl:
                  for gi, lo, co, gw, ps, sg, uv, th, psg in gdat:
                    nc.scalar.activation(th[64:89, co:co + gw],
                                         gc[gi][32:57, pcur + co:pcur + co + gw], AF.Tanh)
                  for gi, lo, co, gw, ps, sg, uv, th, psg in gdat:
                    nc.vector.tensor_mul(
                        hbuf[0:25, hcol(w, lo):hcol(w, lo) + gw],
                        sg[64:89, co:co + gw], th[64:89, co:co + gw])

                if w >= L - 1 and not _noh9:
                    t9 = w - (L - 1)
                    nc.sync.dma_start(
                        out=h9d[:, t9 * BS:(t9 + 1) * BS],
                        in_=hbuf[0:25, hcol(w, 9):hcol(w, 9) + 64])
                if w % 64 == 0 and (w + 128) < Tn:
                    blk = w // 64 + 2
                    slot = blk % 2
                    nc.sync.dma_start(
                        out=xstage[:, slot * 64 * BS:(slot + 1) * 64 * BS],
                        in_=xT[:, blk * 64 * BS:(blk + 1) * 64 * BS])

        # ---------------- tail: FC + exp + sums
        with (
            tc.tile_pool(name="tailw", bufs=4) as twp,
            tc.tile_pool(name="tps", bufs=2, space="PSUM") as tps,
            tc.tile_pool(name="tpers", bufs=1) as tpers,
        ):
            rhs = [tpers.tile([26, 512], F16, name=f"rhs{i}") for i in range(2)]
            for i in range(2):
                nc.sync.dma_start(out=rhs[i][25:26, :], in_=onesd[:, 0:512])
            for j in range(NGRP):
                fcps = tps.tile([103, 512], F32, tag="fcps")
                for g in range(4):
                    ch = 4 * j + g
                    rt = rhs[ch % 2]
                    nc.sync.dma_start(
                        out=rt[0:25, :], in_=h9d[:, ch * 512:(ch + 1) * 512])
                    nc.tensor.matmul(
                        fcps[0:103, :],
                        wsb[0:26, wcols[("fc", g)]:wcols[("fc", g)] + 103],
                        rt[0:26, :], start=(g == 0), stop=(g == 3))
                esb = twp.tile([103, 512], F16, tag="esb")
                nc.scalar.activation(esb[0:103, :], fcps[0:103, :], AF.Exp)
                sps = tps.tile([4, 512], F32, tag="sps")
                nc.tensor.matmul(sps[0:4, :], selsb[0:103, :], esb[0:103, :],
                                 start=True, stop=True)
                ssb = twp.tile([32, 512], F32, tag="ssb")
                nc.scalar.copy(ssb[0:4, :], sps[0:4, :])
                nc.sync.dma_start(out=exp_out[j, :, :], in_=esb[0:103, :])
                nc.sync.dma_start(out=sum_out[j, :, :], in_=ssb[0:4, :])

    if patch_waits:
        _split_excess_waits(nc)
    return nc


def _get_program(Tn, wcols, nwcols):
    if Tn not in _PROG_CACHE:
        _PROG_CACHE[Tn] = _build_program(Tn, wcols, nwcols)
    return _PROG_CACHE[Tn]


# ---------------------------------------------------------------- kernel
def _make_inputs(x, h0, c0, blob, sel, Tn):
    in_maps = []
    onesd = np.ones((1, NB * 640), np.float16)
    for c in range(NC):
        sl = slice(c * BS, (c + 1) * BS)
        xt = np.empty((26, Tn * BS), np.float16)
        xt[0:25] = x[:Tn, sl, :].transpose(2, 0, 1).reshape(25, -1)
        xt[25] = 1.0
        c0t = np.empty((25, L * BS), np.float16)
        for l in range(L):
            c0t[:, l * BS:(l + 1) * BS] = c0[l, sl, :].T
        h0t = np.ascontiguousarray(h0[:, sl, :].transpose(0, 2, 1)).astype(np.float16)
        in_maps.append({"xT": xt, "wpack": blob, "selp": sel, "c0T": c0t,
                        "h0T": h0t, "onesd": onesd})
    return in_maps


def _assemble(results, Tn):
    y = np.empty((Tn, B, NCLS), np.float32)
    for c in range(NC):
        e = results[c]["exp_out"].astype(np.float32)   # [NGRP, 103, 512]
        s = results[c]["sum_out"]                      # [NGRP, 4, 512]
        NGRP = e.shape[0]
        yc = np.empty((Tn * BS, NCLS), np.float32)
        for g in range(4):
            bands = e[:, 32 * g:32 * g + 7, :]
            sums = s[:, g, :]
            vals = (bands / sums[:, None, :]).transpose(0, 2, 1)
            for jj in range(NGRP):
                ch = jj * 4 + g
                yc[ch * 512:(ch + 1) * 512] = vals[jj]
        y[:, c * BS:(c + 1) * BS, :] = yc.reshape(Tn, BS, NCLS)
    return y.reshape(Tn * B, NCLS)


def kernel(x, h0, c0, W_ih, W_hh, b, fc_w, fc_b, _trace=False, _Tn=None):
    x = np.asarray(x, np.float32)
    h0 = np.asarray(h0, np.float32)
    c0 = np.asarray(c0, np.float32)
    Tn = x.shape[0] if _Tn is None else _Tn

    blob, wcols, sel = _pack_weights(
        np.asarray(W_ih, np.float32), np.asarray(W_hh, np.float32),
        np.asarray(b, np.float32), np.asarray(fc_w, np.float32),
        np.asarray(fc_b, np.float32))

    nc = _get_program(Tn, wcols, blob.shape[1])
    in_maps = _make_inputs(x, h0, c0, blob, sel, Tn)
    res = run_bass_kernel_spmd(nc, in_maps, list(range(NC)), trace=_trace)
    out = _assemble(res.results, Tn)
    return (out, res) if _trace else out


if __name__ == "__main__":
    pass
